# revision 50
# baseline (speedup 1.0000x reference)
"""Trainium2 Bass kernel for nn_MultiHeadAttention (N=2048, D=1024, H=16, causal).

Sharding: 16 heads split across 8 NeuronCores (2 heads/core, tensor-parallel
on the head dim).  Each core projects Q^T/K^T (its 128 head-dims x full
sequence) and V for its heads, computes causal attention, applies its
128-row slice of Wo, and writes a bf16 partial [2048, 1024] output.  The
host sums the 8 partials and adds bo.

v2 design (cost-model driven):
  - all matmul operands bf16 (halves HBM traffic vs fp32; 1 cycle/row on PE
    regardless of free size).  End-to-end rel err ~4e-3 vs fp32 reference.
  - scores computed transposed ([nk, nq]) with block-causal trimming; exp on
    the Act engine over PAIRS of nk-blocks (fewer, larger activations);
    triangular masking of diagonal blocks via gpsimd affine_select.
  - PV in [nq, dl] orientation: lhsT = probs block [nk, nq], rhs = Vaug
    [nk, 65] (64 v-dims + ones column -> denominator falls out in col 64).
    Block-exact causal trimming; per-partition (per-row) softmax
    normalization via DVE reciprocal + tensor_scalar_mul.
  - attn [nq, dl] -> PE-transpose -> attnT for the Wo projection.
  - inputs DMA'd in large grouped chunks (quarter-column k/q groups so the
    first score tile starts ~7us in); bf16 output staged via DVE/Pool
    copies, DMA'd from SBUF.
"""
import os
import sys

for _p in ("/opt/trn_rl_repo", "/root/.axon_site/_ro/trn_rl_repo"):
    if os.path.isdir(_p) and _p not in sys.path:
        sys.path.append(_p)

import numpy as np

import concourse.bass as bass
import concourse.mybir as mybir
from concourse import bacc
from concourse.bass_utils import run_bass_kernel_spmd
from concourse.tile import TileContext
from concourse.masks import make_identity
from contextlib import ExitStack

N = 2048
D = 1024
NCORES = 8
DL = 128          # head-dims per core (2 heads x 64)
DK = 64

F32 = mybir.dt.float32
BF16 = mybir.dt.bfloat16


def build_nc(opts=None):
    o = dict(out_copy_split=True)
    if opts:
        o.update(opts)
    nc = bacc.Bacc("TRN2", target_bir_lowering=False, debug=False,
                   num_devices=NCORES)

    qT = nc.dram_tensor("qT", [D, N], BF16, kind="ExternalInput")
    kT = nc.dram_tensor("kT", [D, N], BF16, kind="ExternalInput")
    vT = nc.dram_tensor("vT", [D, N], BF16, kind="ExternalInput")
    wqT = nc.dram_tensor("wqT", [DL, 8 * DL], BF16, kind="ExternalInput")
    wkT = nc.dram_tensor("wkT", [DL, 8 * DL], BF16, kind="ExternalInput")
    wvT = nc.dram_tensor("wvT", [DL, 8 * DL], BF16, kind="ExternalInput")
    bqk = nc.dram_tensor("bqk", [DL, 2], F32, kind="ExternalInput")
    bvrow = nc.dram_tensor("bvrow", [1, DL], BF16, kind="ExternalInput")
    woT = nc.dram_tensor("woT", [DL, D], BF16, kind="ExternalInput")
    out = nc.dram_tensor("out", [N, D], BF16, kind="ExternalOutput")

    AF = mybir.ActivationFunctionType

    with TileContext(nc) as tc, ExitStack() as ctx:
        const = ctx.enter_context(tc.tile_pool(name="const", bufs=1))
        big = ctx.enter_context(tc.tile_pool(name="big", bufs=1))
        kqs = ctx.enter_context(tc.tile_pool(name="kqs", bufs=6))
        vs = ctx.enter_context(tc.tile_pool(name="vs", bufs=2))
        probs_pool = ctx.enter_context(tc.tile_pool(name="probs", bufs=36))
        rc_pool = ctx.enter_context(tc.tile_pool(name="rc", bufs=4))
        asb_pool = ctx.enter_context(tc.tile_pool(name="asb", bufs=6))
        ob_pool = ctx.enter_context(tc.tile_pool(name="ob", bufs=4))

        # ---- DMA issue order = arrival order: wk, k0, wq, q0, k1, q1,
        # [small consts], k2, q2, k3, q3, v0, v1.  k/q land in full
        # column-quarter tiles [128, 8j, 512]; v in half tiles [128, 8j, 1024].
        kgrp = {}
        qgrp = {}

        def load_kq(qt, src, grp, split=False):
            t_ = kqs.tile([128, 8, 512], BF16, name="kq")
            if split:  # two DMAs so the first j-chunks land sooner
                for g in range(2):
                    nc.sync.dma_start(
                        t_[:, 4 * g:4 * (g + 1), :],
                        src[512 * g:512 * (g + 1),
                            512 * qt:512 * (qt + 1)].rearrange(
                                "(j p) n -> p j n", p=128))
            else:
                nc.sync.dma_start(
                    t_[:],
                    src[:, 512 * qt:512 * (qt + 1)].rearrange(
                        "(j p) n -> p j n", p=128))
            grp[qt] = t_

        wk = const.tile([128, 8, DL], BF16)
        nc.sync.dma_start(wk[:], wkT.rearrange("p (j d) -> p j d", j=8))
        load_kq(0, kT, kgrp, split=True)
        wq = const.tile([128, 8, DL], BF16)
        nc.sync.dma_start(wq[:], wqT.rearrange("p (j d) -> p j d", j=8))
        bias_qk = const.tile([128, 2], F32)
        nc.sync.dma_start(bias_qk[:], bqk[:])
        load_kq(0, qT, qgrp, split=True)
        load_kq(1, kT, kgrp)
        load_kq(1, qT, qgrp)
        wv = const.tile([128, 8, DL], BF16)
        nc.sync.dma_start(wv[:], wvT.rearrange("p (j d) -> p j d", j=8))
        wo = const.tile([128, D], BF16)
        nc.sync.dma_start(wo[:], woT[:])
        bv_row = const.tile([1, DL], BF16)
        nc.sync.dma_start(bv_row[:], bvrow[:])
        load_kq(2, kT, kgrp)
        load_kq(2, qT, qgrp)
        load_kq(3, kT, kgrp)
        load_kq(3, qT, qgrp)
        vgrp = {}
        for hf in range(2):
            t_ = vs.tile([128, 8, 1024], BF16, name="vg")
            for g in range(2):  # split by sequence half: first n-blocks land
                nc.sync.dma_start(  # ~3us sooner, unblocking attention t0
                    t_[:, :, 512 * g:512 * (g + 1)],
                    vT[:, 1024 * hf + 512 * g:
                       1024 * hf + 512 * (g + 1)].rearrange(
                        "(j p) n -> p j n", p=128))
            vgrp[hf] = t_

        ones_n = const.tile([1, 128], BF16)
        nc.vector.memset(ones_n[:], 1.0)
        ident = const.tile([128, 128], BF16)
        make_identity(nc, ident[:])

        QT = big.tile([128, N], BF16)
        KT = big.tile([128, N], BF16)
        attnT = big.tile([128, N], BF16)
        Vaug = big.tile([128, 16, 2, 65], BF16)
        nc.vector.memset(Vaug[:, :, :, 64:65], 1.0)

        P = {}
        with tc.tile_pool(name="scps", bufs=2, space="PSUM") as scps, \
             ExitStack() as psctx:
            P["m"] = psctx.enter_context(
                tc.tile_pool(name="mpsA", bufs=1, space="PSUM"))
            P["pvq"] = psctx.enter_context(
                tc.tile_pool(name="pvqA", bufs=2, space="PSUM"))
            P["tp"] = psctx.enter_context(
                tc.tile_pool(name="tpA", bufs=1, space="PSUM"))

            def proj_qk(t):
                for src, w, bcol, dst in ((kgrp, wk, 1, KT), (qgrp, wq, 0, QT)):
                    ps = P["m"].tile([128, 512], F32, name="m")
                    for j in range(8):
                        nc.tensor.matmul(ps[:], w[:, j, :],
                                         src[t][:, j, :],
                                         start=(j == 0), stop=(j == 7))
                    nc.vector.tensor_scalar_add(dst[:, 512 * t:512 * (t + 1)],
                                                ps[:],
                                                bias_qk[:, bcol:bcol + 1])

            def vproj_block(b):
                # V projection for n-block b in [n, dl] layout + bias.
                hf = b // 8
                ps = P["m"].tile([128, 512], F32, name="m")
                for j in range(8):
                    nc.tensor.matmul(ps[:, 0:128],
                                     vgrp[hf][:, j,
                                              128 * (b % 8):
                                              128 * (b % 8) + 128],
                                     wv[:, j, :],
                                     start=(j == 0), stop=False)
                nc.tensor.matmul(ps[:, 0:128], ones_n[:], bv_row[:],
                                 start=False, stop=True)
                nc.vector.tensor_copy(
                    Vaug[:, b, :, 0:64],
                    ps[:, 0:128].rearrange("p (h d) -> p h d", h=2))

            # probs tiles, keyed by (t, h, pair): cols = compacted widths
            probs = {}

            def scores_pair(t, h, pr):
                """Scores + exp + causal mask for nk-blocks (2*pr, 2*pr+1)."""
                b0 = 2 * pr
                off = [max(0, 128 * (b0 + i - 4 * t)) for i in range(2)]
                w = [512 - off[i] for i in range(2)]
                sc2 = scps.tile([128, 1024], F32, name="sc2")
                for i in range(2):
                    b = b0 + i
                    c0 = 0 if i == 0 else w[0]
                    nc.tensor.matmul(
                        sc2[:, c0:c0 + w[i]],
                        KT[64 * h:64 * (h + 1), 128 * b:128 * (b + 1)],
                        QT[64 * h:64 * (h + 1),
                           512 * t + off[i]:512 * (t + 1)],
                        start=True, stop=True)
                pt = probs_pool.tile([128, 1024], BF16, name="probs")
                nc.scalar.activation(pt[:, 0:w[0] + w[1]],
                                     sc2[:, 0:w[0] + w[1]], AF.Exp,
                                     scale=0.125)
                for i in range(2):
                    b = b0 + i
                    if b >= 4 * t:  # diagonal block: triangular mask
                        c0 = 0 if i == 0 else w[0]
                        nc.gpsimd.affine_select(
                            out=pt[:, c0:c0 + 128],
                            in_=pt[:, c0:c0 + 128],
                            compare_op=mybir.AluOpType.is_ge, fill=0.0,
                            base=0, pattern=[[1, 128]],
                            channel_multiplier=-1)
                probs[(t, h, pr)] = (pt, off, w)

            def copy_dve(dst, src):
                nc.vector.tensor_copy(dst, src)

            def copy_act(dst, src):
                nc.scalar.activation(dst, src, AF.Copy)

            copy_engines = (copy_dve, copy_act) if o["out_copy_split"] \
                else (copy_dve, copy_dve)

            pending = []   # software pipeline: (m, asb) awaiting transpose

            def attention_sub(t, sub):
                """PV + normalize for nq block m = 4t+sub; transpose and the
                Wo projection are deferred one sub (flush_one) so the PE
                never waits on the DVE recip/mul chain in program order."""
                m = 4 * t + sub
                pvq = P["pvq"].tile([128, 2, 68], F32, name="pvq")
                for h in range(2):
                    for b in range(m + 1):
                        pt, off, w = probs[(t, h, b // 2)]
                        c0 = (0 if b % 2 == 0 else w[0]) \
                            + 128 * sub - off[b % 2]
                        nc.tensor.matmul(pvq[:, h, 0:65],
                                         pt[:, c0:c0 + 128],
                                         Vaug[:, b, h, 0:65],
                                         start=(b == 0), stop=(b == m))
                rc = rc_pool.tile([128, 2], F32, name="rc")
                nc.vector.reciprocal(rc[:], pvq[:, :, 64:65])
                asb = asb_pool.tile([128, 128], BF16, name="asb")
                for h in range(2):
                    nc.vector.tensor_scalar_mul(asb[:, 64 * h:64 * (h + 1)],
                                                pvq[:, h, 0:64],
                                                rc[:, h:h + 1])
                pending.append((m, asb))

            def flush_one(force=False):
                # keep one sub in flight: pop the PREVIOUS sub so its
                # transpose never waits on the just-issued DVE chain
                if len(pending) < (1 if force else 4):
                    return
                m, asb = pending.pop(0)
                tp = P["tp"].tile([128, 128], BF16, name="tp")
                nc.tensor.transpose(tp[:], asb[:], ident[:])
                nc.vector.tensor_copy(attnT[:, 128 * m:128 * (m + 1)], tp[:])
                out_proj(m)

            def out_proj(m):
                ob = ob_pool.tile([128, 1024], BF16, name="ob")
                for u in range(2):
                    wps = P["m"].tile([128, 512], F32, name="m")
                    nc.tensor.matmul(wps[:],
                                     attnT[:, 128 * m:128 * (m + 1)],
                                     wo[:, 512 * u:512 * (u + 1)],
                                     start=True, stop=True)
                    copy_engines[u](ob[:, 512 * u:512 * (u + 1)], wps[:])
                nc.sync.dma_start(out[128 * m:128 * (m + 1), :], ob[:])

            # ---------------- schedule ----------------
            proj_qk(0)
            for h in range(2):
                for pr in range(2):
                    scores_pair(0, h, pr)
            proj_qk(1)
            for h in range(2):
                for pr in range(4):
                    scores_pair(1, h, pr)
            proj_qk(2)
            for h in range(2):
                for pr in range(6):
                    scores_pair(2, h, pr)
            proj_qk(3)
            # t3 scores interleaved with V projection + early attention so
            # the PE keeps busy while the Act engine drains the exp backlog.
            t3_pairs = [(h, pr) for pr in range(8) for h in range(2)]
            for i, (h, pr) in enumerate(t3_pairs[:6]):
                scores_pair(3, h, pr)
                if i >= 2:
                    vproj_block(i - 2)      # b0..b3
            for b in range(4, 8):
                vproj_block(b)
            for sub in range(4):            # t=0 attention
                attention_sub(0, sub)
                flush_one()
                scores_pair(3, *t3_pairs[6 + sub])
            for sub in range(4):            # t=1 attention
                attention_sub(1, sub)
                flush_one()
                scores_pair(3, *t3_pairs[10 + sub])
            scores_pair(3, *t3_pairs[14])
            scores_pair(3, *t3_pairs[15])

        # ---- phase B: scores done; re-pool PSUM with deeper buffering ----
        with tc.tile_pool(name="mpsB", bufs=4, space="PSUM") as mB, \
             tc.tile_pool(name="pvqB", bufs=2, space="PSUM") as pvqB, \
             tc.tile_pool(name="tpB", bufs=2, space="PSUM") as tpB:
            P["m"], P["pvq"], P["tp"] = mB, pvqB, tpB
            vproj_block(8)
            for sub in range(4):            # t=2 attention
                vproj_block(9 + sub)
                attention_sub(2, sub)
                flush_one()
            vproj_block(13)
            for sub in range(4):            # t=3 attention
                if sub < 2:
                    vproj_block(14 + sub)
                attention_sub(3, sub)
                flush_one()
                if sub >= 2:    # taper: drain early so the final tail
                    flush_one(force=True)  # holds only one flush chain
            for _ in range(4):      # exhaustive drain (no-op once empty)
                flush_one(force=True)

    nc.compile()
    return nc


def make_in_maps(q, k, v, Wq, bq, Wk, bk, Wv, bv, Wo, bo):
    import ml_dtypes
    bf = ml_dtypes.bfloat16
    f32 = np.float32
    qTa = np.ascontiguousarray(q.T).astype(bf)
    kTa = np.ascontiguousarray(k.T).astype(bf)
    vTa = np.ascontiguousarray(v.T).astype(bf)
    WqT = np.ascontiguousarray(Wq.T)
    WkT = np.ascontiguousarray(Wk.T)
    WvT = np.ascontiguousarray(Wv.T)
    WoT = np.ascontiguousarray(Wo.T)

    def pack_w(WT, d0):
        # [1024, 128] slice -> [128, 8*128]: row p holds chunks j at
        # [128j + p, :] so SBUF tile [128, 8, 128] has [:, j, :] = chunk j.
        sl = WT[:, d0:d0 + DL]                     # [1024, 128]
        return np.ascontiguousarray(
            sl.reshape(8, 128, DL).transpose(1, 0, 2).reshape(128, 8 * DL)
        ).astype(bf)

    in_maps = []
    for c in range(NCORES):
        d0 = DL * c
        in_maps.append({
            "qT": qTa, "kT": kTa, "vT": vTa,
            "wqT": pack_w(WqT, d0),
            "wkT": pack_w(WkT, d0),
            "wvT": pack_w(WvT, d0),
            "bqk": np.ascontiguousarray(
                np.stack([bq[d0:d0 + DL], bk[d0:d0 + DL]], axis=1)
            ).astype(f32),
            "bvrow": bv[d0:d0 + DL].reshape(1, DL).astype(bf),
            "woT": np.ascontiguousarray(WoT[d0:d0 + DL, :]).astype(bf),
        })
    return in_maps


_NC_CACHE = None


def _get_nc():
    global _NC_CACHE
    if _NC_CACHE is None:
        _NC_CACHE = build_nc()
    return _NC_CACHE


def kernel(q, k, v, Wq, bq, Wk, bk, Wv, bv, Wo, bo):
    """Full-input / full-output entry point (harness contract)."""
    q, k, v = np.asarray(q), np.asarray(k), np.asarray(v)
    Wq, bq, Wk, bk = np.asarray(Wq), np.asarray(bq), np.asarray(Wk), np.asarray(bk)
    Wv, bv, Wo, bo = np.asarray(Wv), np.asarray(bv), np.asarray(Wo), np.asarray(bo)
    nc = _get_nc()
    in_maps = make_in_maps(q, k, v, Wq, bq, Wk, bk, Wv, bv, Wo, bo)
    res = run_bass_kernel_spmd(nc, in_maps, list(range(NCORES)))
    acc = res.results[0]["out"].astype(np.float64)
    for c in range(1, NCORES):
        acc += res.results[c]["out"].astype(np.float64)
    acc += bo.astype(np.float64)
    return acc.astype(np.float32)


# revision 51
# speedup vs baseline: 1.0040x; 1.0040x over previous
"""Trainium2 Bass kernel for nn_MultiHeadAttention (N=2048, D=1024, H=16, causal).

Sharding: 16 heads split across 8 NeuronCores (2 heads/core, tensor-parallel
on the head dim).  Each core projects Q^T/K^T (its 128 head-dims x full
sequence) and V for its heads, computes causal attention, applies its
128-row slice of Wo, and writes a bf16 partial [2048, 1024] output.  The
host sums the 8 partials and adds bo.

v2 design (cost-model driven):
  - all matmul operands bf16 (halves HBM traffic vs fp32; 1 cycle/row on PE
    regardless of free size).  End-to-end rel err ~4e-3 vs fp32 reference.
  - scores computed transposed ([nk, nq]) with block-causal trimming; exp on
    the Act engine over PAIRS of nk-blocks (fewer, larger activations);
    triangular masking of diagonal blocks via gpsimd affine_select.
  - PV in [nq, dl] orientation: lhsT = probs block [nk, nq], rhs = Vaug
    [nk, 65] (64 v-dims + ones column -> denominator falls out in col 64).
    Block-exact causal trimming; per-partition (per-row) softmax
    normalization via DVE reciprocal + tensor_scalar_mul.
  - attn [nq, dl] -> PE-transpose -> attnT for the Wo projection.
  - inputs DMA'd in large grouped chunks (quarter-column k/q groups so the
    first score tile starts ~7us in); bf16 output staged via DVE/Pool
    copies, DMA'd from SBUF.
"""
import os
import sys

for _p in ("/opt/trn_rl_repo", "/root/.axon_site/_ro/trn_rl_repo"):
    if os.path.isdir(_p) and _p not in sys.path:
        sys.path.append(_p)

import numpy as np

import concourse.bass as bass
import concourse.mybir as mybir
from concourse import bacc
from concourse.bass_utils import run_bass_kernel_spmd
from concourse.tile import TileContext
from concourse.masks import make_identity
from contextlib import ExitStack

N = 2048
D = 1024
NCORES = 8
DL = 128          # head-dims per core (2 heads x 64)
DK = 64

F32 = mybir.dt.float32
BF16 = mybir.dt.bfloat16


def build_nc(opts=None):
    o = dict(out_copy_split=True)
    if opts:
        o.update(opts)
    nc = bacc.Bacc("TRN2", target_bir_lowering=False, debug=False,
                   num_devices=NCORES)

    qT = nc.dram_tensor("qT", [D, N], BF16, kind="ExternalInput")
    kT = nc.dram_tensor("kT", [D, N], BF16, kind="ExternalInput")
    vT = nc.dram_tensor("vT", [D, N], BF16, kind="ExternalInput")
    wqT = nc.dram_tensor("wqT", [DL, 8 * DL], BF16, kind="ExternalInput")
    wkT = nc.dram_tensor("wkT", [DL, 8 * DL], BF16, kind="ExternalInput")
    wvT = nc.dram_tensor("wvT", [DL, 8 * DL], BF16, kind="ExternalInput")
    bqk = nc.dram_tensor("bqk", [DL, 2], F32, kind="ExternalInput")
    bvrow = nc.dram_tensor("bvrow", [1, DL], BF16, kind="ExternalInput")
    woT = nc.dram_tensor("woT", [DL, D], BF16, kind="ExternalInput")
    out = nc.dram_tensor("out", [N, D], BF16, kind="ExternalOutput")

    AF = mybir.ActivationFunctionType

    with TileContext(nc) as tc, ExitStack() as ctx:
        const = ctx.enter_context(tc.tile_pool(name="const", bufs=1))
        big = ctx.enter_context(tc.tile_pool(name="big", bufs=1))
        kqs = ctx.enter_context(tc.tile_pool(name="kqs", bufs=6))
        vs = ctx.enter_context(tc.tile_pool(name="vs", bufs=2))
        probs_pool = ctx.enter_context(tc.tile_pool(name="probs", bufs=36))
        rc_pool = ctx.enter_context(tc.tile_pool(name="rc", bufs=4))
        asb_pool = ctx.enter_context(tc.tile_pool(name="asb", bufs=6))
        ob_pool = ctx.enter_context(tc.tile_pool(name="ob", bufs=4))

        # ---- DMA issue order = arrival order: wk, k0, wq, q0, k1, q1,
        # [small consts], k2, q2, k3, q3, v0, v1.  k/q land in full
        # column-quarter tiles [128, 8j, 512]; v in half tiles [128, 8j, 1024].
        kgrp = {}
        qgrp = {}

        def load_kq(qt, src, grp, split=False):
            t_ = kqs.tile([128, 8, 512], BF16, name="kq")
            if split:  # two DMAs so the first j-chunks land sooner
                for g in range(2):
                    nc.sync.dma_start(
                        t_[:, 4 * g:4 * (g + 1), :],
                        src[512 * g:512 * (g + 1),
                            512 * qt:512 * (qt + 1)].rearrange(
                                "(j p) n -> p j n", p=128))
            else:
                nc.sync.dma_start(
                    t_[:],
                    src[:, 512 * qt:512 * (qt + 1)].rearrange(
                        "(j p) n -> p j n", p=128))
            grp[qt] = t_

        wk = const.tile([128, 8, DL], BF16)
        nc.sync.dma_start(wk[:], wkT.rearrange("p (j d) -> p j d", j=8))
        load_kq(0, kT, kgrp, split=True)
        wq = const.tile([128, 8, DL], BF16)
        nc.sync.dma_start(wq[:], wqT.rearrange("p (j d) -> p j d", j=8))
        bias_qk = const.tile([128, 2], F32)
        nc.sync.dma_start(bias_qk[:], bqk[:])
        load_kq(0, qT, qgrp, split=True)
        load_kq(1, kT, kgrp)
        load_kq(1, qT, qgrp)
        wv = const.tile([128, 8, DL], BF16)
        nc.sync.dma_start(wv[:], wvT.rearrange("p (j d) -> p j d", j=8))
        wo = const.tile([128, D], BF16)
        nc.sync.dma_start(wo[:], woT[:])
        bv_row = const.tile([1, DL], BF16)
        nc.sync.dma_start(bv_row[:], bvrow[:])
        load_kq(2, kT, kgrp)
        load_kq(2, qT, qgrp)
        load_kq(3, kT, kgrp)
        load_kq(3, qT, qgrp)
        vgrp = {}
        for hf in range(2):
            t_ = vs.tile([128, 8, 1024], BF16, name="vg")
            nc.sync.dma_start(
                t_[:],
                vT[:, 1024 * hf:1024 * (hf + 1)].rearrange(
                    "(j p) n -> p j n", p=128))
            vgrp[hf] = t_

        ones_n = const.tile([1, 128], BF16)
        nc.vector.memset(ones_n[:], 1.0)
        ident = const.tile([128, 128], BF16)
        make_identity(nc, ident[:])

        QT = big.tile([128, N], BF16)
        KT = big.tile([128, N], BF16)
        attnT = big.tile([128, N], BF16)
        Vaug = big.tile([128, 16, 2, 65], BF16)
        nc.vector.memset(Vaug[:, :, :, 64:65], 1.0)

        P = {}
        with tc.tile_pool(name="scps", bufs=2, space="PSUM") as scps, \
             ExitStack() as psctx:
            P["m"] = psctx.enter_context(
                tc.tile_pool(name="mpsA", bufs=1, space="PSUM"))
            P["pvq"] = psctx.enter_context(
                tc.tile_pool(name="pvqA", bufs=2, space="PSUM"))
            P["tp"] = psctx.enter_context(
                tc.tile_pool(name="tpA", bufs=1, space="PSUM"))

            def proj_qk(t):
                for src, w, bcol, dst in ((kgrp, wk, 1, KT), (qgrp, wq, 0, QT)):
                    ps = P["m"].tile([128, 512], F32, name="m")
                    for j in range(8):
                        nc.tensor.matmul(ps[:], w[:, j, :],
                                         src[t][:, j, :],
                                         start=(j == 0), stop=(j == 7))
                    nc.vector.tensor_scalar_add(dst[:, 512 * t:512 * (t + 1)],
                                                ps[:],
                                                bias_qk[:, bcol:bcol + 1])

            def vproj_block(b):
                # V projection for n-block b in [n, dl] layout + bias.
                hf = b // 8
                ps = P["m"].tile([128, 512], F32, name="m")
                for j in range(8):
                    nc.tensor.matmul(ps[:, 0:128],
                                     vgrp[hf][:, j,
                                              128 * (b % 8):
                                              128 * (b % 8) + 128],
                                     wv[:, j, :],
                                     start=(j == 0), stop=False)
                nc.tensor.matmul(ps[:, 0:128], ones_n[:], bv_row[:],
                                 start=False, stop=True)
                nc.vector.tensor_copy(
                    Vaug[:, b, :, 0:64],
                    ps[:, 0:128].rearrange("p (h d) -> p h d", h=2))

            # probs tiles, keyed by (t, h, pair): cols = compacted widths
            probs = {}

            def scores_pair(t, h, pr):
                """Scores + exp + causal mask for nk-blocks (2*pr, 2*pr+1)."""
                b0 = 2 * pr
                off = [max(0, 128 * (b0 + i - 4 * t)) for i in range(2)]
                w = [512 - off[i] for i in range(2)]
                sc2 = scps.tile([128, 1024], F32, name="sc2")
                for i in range(2):
                    b = b0 + i
                    c0 = 0 if i == 0 else w[0]
                    nc.tensor.matmul(
                        sc2[:, c0:c0 + w[i]],
                        KT[64 * h:64 * (h + 1), 128 * b:128 * (b + 1)],
                        QT[64 * h:64 * (h + 1),
                           512 * t + off[i]:512 * (t + 1)],
                        start=True, stop=True)
                pt = probs_pool.tile([128, 1024], BF16, name="probs")
                nc.scalar.activation(pt[:, 0:w[0] + w[1]],
                                     sc2[:, 0:w[0] + w[1]], AF.Exp,
                                     scale=0.125)
                for i in range(2):
                    b = b0 + i
                    if b >= 4 * t:  # diagonal block: triangular mask
                        c0 = 0 if i == 0 else w[0]
                        nc.gpsimd.affine_select(
                            out=pt[:, c0:c0 + 128],
                            in_=pt[:, c0:c0 + 128],
                            compare_op=mybir.AluOpType.is_ge, fill=0.0,
                            base=0, pattern=[[1, 128]],
                            channel_multiplier=-1)
                probs[(t, h, pr)] = (pt, off, w)

            def copy_dve(dst, src):
                nc.vector.tensor_copy(dst, src)

            def copy_act(dst, src):
                nc.scalar.activation(dst, src, AF.Copy)

            copy_engines = (copy_dve, copy_act) if o["out_copy_split"] \
                else (copy_dve, copy_dve)

            pending = []   # software pipeline: (m, asb) awaiting transpose

            def attention_sub(t, sub):
                """PV + normalize for nq block m = 4t+sub; transpose and the
                Wo projection are deferred one sub (flush_one) so the PE
                never waits on the DVE recip/mul chain in program order."""
                m = 4 * t + sub
                pvq = P["pvq"].tile([128, 2, 68], F32, name="pvq")
                for h in range(2):
                    for b in range(m + 1):
                        pt, off, w = probs[(t, h, b // 2)]
                        c0 = (0 if b % 2 == 0 else w[0]) \
                            + 128 * sub - off[b % 2]
                        nc.tensor.matmul(pvq[:, h, 0:65],
                                         pt[:, c0:c0 + 128],
                                         Vaug[:, b, h, 0:65],
                                         start=(b == 0), stop=(b == m))
                rc = rc_pool.tile([128, 2], F32, name="rc")
                nc.vector.reciprocal(rc[:], pvq[:, :, 64:65])
                asb = asb_pool.tile([128, 128], BF16, name="asb")
                for h in range(2):
                    nc.vector.tensor_scalar_mul(asb[:, 64 * h:64 * (h + 1)],
                                                pvq[:, h, 0:64],
                                                rc[:, h:h + 1])
                pending.append((m, asb))

            def flush_one(force=False):
                # keep one sub in flight: pop the PREVIOUS sub so its
                # transpose never waits on the just-issued DVE chain
                if len(pending) < (1 if force else 4):
                    return
                m, asb = pending.pop(0)
                tp = P["tp"].tile([128, 128], BF16, name="tp")
                nc.tensor.transpose(tp[:], asb[:], ident[:])
                nc.vector.tensor_copy(attnT[:, 128 * m:128 * (m + 1)], tp[:])
                out_proj(m)

            def out_proj(m):
                ob = ob_pool.tile([128, 1024], BF16, name="ob")
                for u in range(2):
                    wps = P["m"].tile([128, 512], F32, name="m")
                    nc.tensor.matmul(wps[:],
                                     attnT[:, 128 * m:128 * (m + 1)],
                                     wo[:, 512 * u:512 * (u + 1)],
                                     start=True, stop=True)
                    # early tiles: keep Act free for the exp backlog
                    ce = copy_dve if (u == 1 and m < 8) else copy_engines[u]
                    ce(ob[:, 512 * u:512 * (u + 1)], wps[:])
                nc.sync.dma_start(out[128 * m:128 * (m + 1), :], ob[:])

            # ---------------- schedule ----------------
            proj_qk(0)
            for h in range(2):
                for pr in range(2):
                    scores_pair(0, h, pr)
            proj_qk(1)
            for h in range(2):
                for pr in range(4):
                    scores_pair(1, h, pr)
            proj_qk(2)
            for h in range(2):
                for pr in range(6):
                    scores_pair(2, h, pr)
            proj_qk(3)
            # t3 scores interleaved with V projection + early attention so
            # the PE keeps busy while the Act engine drains the exp backlog.
            t3_pairs = [(h, pr) for pr in range(8) for h in range(2)]
            for i, (h, pr) in enumerate(t3_pairs[:6]):
                scores_pair(3, h, pr)
                if i >= 2:
                    vproj_block(i - 2)      # b0..b3
            for b in range(4, 8):
                vproj_block(b)
            for sub in range(4):            # t=0 attention
                attention_sub(0, sub)
                flush_one()
                scores_pair(3, *t3_pairs[6 + sub])
            for sub in range(4):            # t=1 attention
                attention_sub(1, sub)
                flush_one()
                scores_pair(3, *t3_pairs[10 + sub])
            scores_pair(3, *t3_pairs[14])
            scores_pair(3, *t3_pairs[15])

        # ---- phase B: scores done; re-pool PSUM with deeper buffering ----
        with tc.tile_pool(name="mpsB", bufs=4, space="PSUM") as mB, \
             tc.tile_pool(name="pvqB", bufs=2, space="PSUM") as pvqB, \
             tc.tile_pool(name="tpB", bufs=2, space="PSUM") as tpB:
            P["m"], P["pvq"], P["tp"] = mB, pvqB, tpB
            vproj_block(8)
            for sub in range(4):            # t=2 attention
                vproj_block(9 + sub)
                attention_sub(2, sub)
                flush_one()
            vproj_block(13)
            for sub in range(4):            # t=3 attention
                if sub < 2:
                    vproj_block(14 + sub)
                attention_sub(3, sub)
                flush_one()
                if sub >= 2:    # taper: drain early so the final tail
                    flush_one(force=True)  # holds only one flush chain
            for _ in range(4):      # exhaustive drain (no-op once empty)
                flush_one(force=True)

    nc.compile()
    return nc


def make_in_maps(q, k, v, Wq, bq, Wk, bk, Wv, bv, Wo, bo):
    import ml_dtypes
    bf = ml_dtypes.bfloat16
    f32 = np.float32
    qTa = np.ascontiguousarray(q.T).astype(bf)
    kTa = np.ascontiguousarray(k.T).astype(bf)
    vTa = np.ascontiguousarray(v.T).astype(bf)
    WqT = np.ascontiguousarray(Wq.T)
    WkT = np.ascontiguousarray(Wk.T)
    WvT = np.ascontiguousarray(Wv.T)
    WoT = np.ascontiguousarray(Wo.T)

    def pack_w(WT, d0):
        # [1024, 128] slice -> [128, 8*128]: row p holds chunks j at
        # [128j + p, :] so SBUF tile [128, 8, 128] has [:, j, :] = chunk j.
        sl = WT[:, d0:d0 + DL]                     # [1024, 128]
        return np.ascontiguousarray(
            sl.reshape(8, 128, DL).transpose(1, 0, 2).reshape(128, 8 * DL)
        ).astype(bf)

    in_maps = []
    for c in range(NCORES):
        d0 = DL * c
        in_maps.append({
            "qT": qTa, "kT": kTa, "vT": vTa,
            "wqT": pack_w(WqT, d0),
            "wkT": pack_w(WkT, d0),
            "wvT": pack_w(WvT, d0),
            "bqk": np.ascontiguousarray(
                np.stack([bq[d0:d0 + DL], bk[d0:d0 + DL]], axis=1)
            ).astype(f32),
            "bvrow": bv[d0:d0 + DL].reshape(1, DL).astype(bf),
            "woT": np.ascontiguousarray(WoT[d0:d0 + DL, :]).astype(bf),
        })
    return in_maps


_NC_CACHE = None


def _get_nc():
    global _NC_CACHE
    if _NC_CACHE is None:
        _NC_CACHE = build_nc()
    return _NC_CACHE


def kernel(q, k, v, Wq, bq, Wk, bk, Wv, bv, Wo, bo):
    """Full-input / full-output entry point (harness contract)."""
    q, k, v = np.asarray(q), np.asarray(k), np.asarray(v)
    Wq, bq, Wk, bk = np.asarray(Wq), np.asarray(bq), np.asarray(Wk), np.asarray(bk)
    Wv, bv, Wo, bo = np.asarray(Wv), np.asarray(bv), np.asarray(Wo), np.asarray(bo)
    nc = _get_nc()
    in_maps = make_in_maps(q, k, v, Wq, bq, Wk, bk, Wv, bv, Wo, bo)
    res = run_bass_kernel_spmd(nc, in_maps, list(range(NCORES)))
    acc = res.results[0]["out"].astype(np.float64)
    for c in range(1, NCORES):
        acc += res.results[c]["out"].astype(np.float64)
    acc += bo.astype(np.float64)
    return acc.astype(np.float32)
